# revision 49
# baseline (speedup 1.0000x reference)
"""Trainium2 Bass kernel for nn_CAModel (neural cellular automaton step).

v4 strategy (data-parallel over batch, 16 samples -> 8 cores x 2):
  - w-direction sobel taps folded into mm1's contraction dim (K=97):
    S rows = [x(16); V(w-1); V(w+1); D(w-1); D(w); D(w+1); ones], with
    V = [1,2,1]_h smoothing, D = x(h+1)-x(h-1), ones carrying b1.
  - S staging lives in padded pixel space (pitch 258): all shift gathers
    are contiguous; pixel tiles (128) never straddle a row.
  - 16-consecutive-partition SBUF->SBUF DMAs only reach 4/16 SDMA ports,
    so V/D round-trip through DRAM: one wide store each, then fast
    DRAM->SBUF block gathers (x rows gather straight from the xbf DRAM
    copy).  DRAM scratch comes from a DRAM tile pool so store->gather
    RAW hazards are tracked.
  - relu evac = max(psum,0) split between ScalarE ACTIVATE and VectorE
    TENSOR_SCALAR; the Scalar ring issues no DMAs at all.
  - x bf16 end-to-end; per-strip hsb/psdx; bank evac + masks on Vector;
    pool neighbor staging entirely by DMA.
Host does layout transforms only; HW exec time is what's measured.
"""

import numpy as np

# ---------------------------------------------------------------- constants
B, C, H, W = 16, 16, 256, 256
NCORES = 8
SPC = B // NCORES
HWPX = H * W
PITCH = 258
NROWH = 34
XBF_F = NROWH * PITCH      # 8772
SOB_F = 32 * PITCH         # 8256
PIX_F = 8192
NT = HWPX // 128           # 512
NSTRIP = 8
KROWS = 97
ALPHA_TH = 0.1
FIRE = 0.5

CHUNKS = [(0, 1536), (1536, 1536), (3072, 1536), (4608, 1536),
          (6144, 1536), (7680, 576)]

# ------------------------------------------------------------ tuning knobs
S_FP8 = True          # fp8e4m3 S staging (w1e scaled x8, evac scales 1/8)
EVAC_V_EVERY = 4      # every Nth chunk's relu evac goes to Vector (rest S)
MULT_PAT = "VV"       # final x*life halves (GpSimd TT is ~2.5x slower)

_BUILT = None


# ------------------------------------------------------------- host layouts
def _bf16():
    import ml_dtypes
    return ml_dtypes.bfloat16


def _fp8():
    import ml_dtypes
    return ml_dtypes.float8_e4m3fn


def _pad_wrap(a):
    out = np.empty(a.shape[:-1] + (PITCH,), dtype=a.dtype)
    out[..., 1:257] = a
    out[..., 0] = a[..., 255]
    out[..., 257] = a[..., 0]
    return out


def _strip_rows(x):
    """x [B,C,H,W] -> [B, 8, C, 34, 258] with halo rows and wrap cols."""
    hidx = (np.arange(-1, 33)[None, :] + 32 * np.arange(8)[:, None]) % 256
    xr = x[:, :, hidx, :]                                  # [B, C, 8, 34, W]
    return _pad_wrap(np.transpose(xr, (0, 2, 1, 3, 4)))


def _prep_xbf(x, dtype):
    out = _strip_rows(x).astype(dtype)
    return np.ascontiguousarray(out.reshape(B, 128, XBF_F))


def _prep_xt(x):
    bf16 = _bf16()
    xf = x.reshape(B, C, HWPX).transpose(0, 2, 1)
    xf = xf.reshape(B, NT, 128, C).transpose(0, 2, 1, 3)
    return np.ascontiguousarray(xf.reshape(B, 128, NT * C).astype(bf16))


def _prep_randt(rv):
    rf = rv.reshape(B, HWPX).reshape(B, NT, 128).transpose(0, 2, 1)
    return np.ascontiguousarray(rf.astype(np.float32))


def _unprep_out(op):
    o = op.astype(np.float32).reshape(B, 128, NT, C).transpose(0, 2, 1, 3)
    o = o.reshape(B, HWPX, C).transpose(0, 2, 1)
    return np.ascontiguousarray(o.reshape(B, C, H, W))


def _prep_weights(w1, b1, w2, b2):
    bf16 = _bf16()
    w1 = np.asarray(w1, np.float32)
    w2 = np.asarray(w2, np.float32)
    wid, wdx, wdy = w1[0::3], w1[1::3], w1[2::3]
    w1e = np.concatenate([
        wid,
        -0.125 * wdx,          # V(w-1)
        0.125 * wdx,           # V(w+1)
        0.125 * wdy,           # D(w-1)
        0.25 * wdy,            # D(w)
        0.125 * wdy,           # D(w+1)
        np.asarray(b1, np.float32).reshape(1, 128),
    ], axis=0)                                            # [97, 128]
    if S_FP8:
        w1e = np.ascontiguousarray((8.0 * w1e).astype(_fp8()))
    else:
        w1e = np.ascontiguousarray(w1e.astype(bf16))
    return (w1e,
            np.ascontiguousarray(w2.astype(bf16)),
            np.asarray(b2, np.float32).reshape(1, 16))


# ------------------------------------------------------------- build module
def _build(b2_nonzero):
    import concourse.bass as bass
    import concourse.bacc as bacc
    import concourse.mybir as mybir
    import concourse.tile as tile

    dt = mybir.dt
    op = mybir.AluOpType
    AF = mybir.ActivationFunctionType
    sdt = dt.float8e4 if S_FP8 else dt.bfloat16

    nc = bacc.Bacc("TRN2", target_bir_lowering=False, debug=False)

    xbf_d = nc.dram_tensor("xbf", (SPC, 128, XBF_F), dt.bfloat16, kind="ExternalInput")
    xg_d = (nc.dram_tensor("xf8", (SPC, 128, XBF_F), sdt, kind="ExternalInput")
            if S_FP8 else xbf_d)
    xt_d = nc.dram_tensor("xt", (SPC, 128, PIX_F), dt.bfloat16, kind="ExternalInput")
    rt_d = nc.dram_tensor("rt", (SPC, 128, NT), dt.float32, kind="ExternalInput")
    w1_d = nc.dram_tensor("w1e", (KROWS, 128), sdt, kind="ExternalInput")
    w2_d = nc.dram_tensor("w2e", (128, 16), dt.bfloat16, kind="ExternalInput")
    ones_d = nc.dram_tensor("onesr", (1, SOB_F), sdt, kind="ExternalInput")
    b2_d = nc.dram_tensor("b2e", (1, 16), dt.float32, kind="ExternalInput")
    out_d = nc.dram_tensor("outp", (SPC, 128, PIX_F), dt.bfloat16, kind="ExternalOutput")

    def eng(name):
        return {"V": nc.vector, "P": nc.gpsimd}[name]

    with tile.TileContext(nc) as tc:
        with (
            tc.tile_pool(name="wpool", bufs=1) as wpool,
            tc.tile_pool(name="xbf", bufs=1) as p_xbf,
            tc.tile_pool(name="pA", bufs=1) as p_A,
            tc.tile_pool(name="pV", bufs=1) as p_V,
            tc.tile_pool(name="pD", bufs=1) as p_D,
            tc.tile_pool(name="xt", bufs=2) as p_xt,
            tc.tile_pool(name="S", bufs=2) as p_S,
            tc.tile_pool(name="hsb", bufs=2) as p_hsb,
            tc.tile_pool(name="small", bufs=2) as p_small,
            tc.tile_pool(name="dx", bufs=2) as p_dx,
            tc.tile_pool(name="pscr", bufs=2) as p_pscr,
            tc.tile_pool(name="vdd", bufs=2, space="DRAM") as p_vdd,
            tc.tile_pool(name="psh", bufs=2, space=bass.MemorySpace.PSUM) as p_psh,
            tc.tile_pool(name="psdx", bufs=1, space=bass.MemorySpace.PSUM) as p_psdx,
        ):
            w1_sb = wpool.tile([KROWS, 128], sdt, tag="w1")
            nc.sync.dma_start(w1_sb[:], w1_d.ap())
            w2_sb = wpool.tile([128, 16], dt.bfloat16, tag="w2")
            nc.sync.dma_start(w2_sb[:], w2_d.ap())
            if b2_nonzero:
                b2_sb = wpool.tile([128, 16], dt.float32, tag="b2")
                nc.sync.dma_start(b2_sb[:], b2_d.ap().broadcast_to([128, 16]))

            # prime both S slots: ones row + edge cols the contiguous
            # shift gathers never write (all persist across slot reuse).
            for _ in range(2):
                St = p_S.tile([KROWS, SOB_F], sdt, tag="S")
                nc.sync.dma_start(St[96:97, :], ones_d.ap())
                nc.vector.memset(St[:, 0:1], 0.0)
                nc.vector.memset(St[:, SOB_F - 1:SOB_F], 0.0)

            def emit_head_loads(s):
                st = {}
                xbf = p_xbf.tile([128, XBF_F], dt.bfloat16, tag="xbf")
                nc.sync.dma_start(xbf[0:64, :], xbf_d.ap()[s, 0:64])
                nc.gpsimd.dma_start(xbf[64:128, :], xbf_d.ap()[s, 64:128])
                xt = p_xt.tile([128, PIX_F], dt.bfloat16, tag="xt")
                nc.gpsimd.dma_start(xt[:], xt_d.ap()[s])
                rt = p_small.tile([128, NT], dt.float32, tag="rt")
                nc.gpsimd.dma_start(rt[:], rt_d.ap()[s])
                xbf3 = xbf.rearrange("p (r q) -> p r q", q=PITCH)
                st.update(xbf3=xbf3, xt=xt, rt=rt,
                          xt3=xt.rearrange("p (t c) -> p t c", c=16))
                return st

            def emit_sobel_a(s, st):
                A = p_A.tile([128, SOB_F], dt.bfloat16, tag="A")
                nc.vector.tensor_tensor(
                    A.rearrange("p (r q) -> p r q", q=PITCH)[:],
                    st["xbf3"][:, 0:32, :], st["xbf3"][:, 2:34, :], op.add)
                st["A"] = A

            def emit_sobel_v(s, st):
                Vt = p_V.tile([128, SOB_F], dt.bfloat16, tag="V")
                nc.vector.scalar_tensor_tensor(
                    Vt.rearrange("p (r q) -> p r q", q=PITCH)[:],
                    st["xbf3"][:, 1:33, :], 2.0,
                    st["A"].rearrange("p (r q) -> p r q", q=PITCH)[:],
                    op.mult, op.add)
                # quarter-stores: strip hb's gather only reads partitions
                # 16hb..16hb+16, so early strips unblock after 1/4 of the
                # (slow, casting) store instead of all of it
                Vd = p_vdd.tile([128, SOB_F], sdt, tag="Vd")
                nc.gpsimd.dma_start(Vd[0:32], Vt[0:32])
                nc.gpsimd.dma_start(Vd[32:64], Vt[32:64])
                st.update(Vd=Vd, Vt=Vt)

            def emit_sobel_d(s, st):
                Dt = p_D.tile([128, SOB_F], dt.bfloat16, tag="D")
                nc.vector.tensor_tensor(
                    Dt.rearrange("p (r q) -> p r q", q=PITCH)[:],
                    st["xbf3"][:, 2:34, :], st["xbf3"][:, 0:32, :],
                    op.subtract)
                Dd = p_vdd.tile([128, SOB_F], sdt, tag="Dd")
                Vd, Vt = st["Vd"], st["Vt"]
                # interleave remaining V and D quarter-stores so strip 0
                # (needs V[0:16]+D[0:16]) unblocks first
                nc.gpsimd.dma_start(Dd[0:32], Dt[0:32])
                nc.gpsimd.dma_start(Vd[64:96], Vt[64:96])
                nc.gpsimd.dma_start(Dd[32:64], Dt[32:64])
                nc.gpsimd.dma_start(Vd[96:128], Vt[96:128])
                nc.gpsimd.dma_start(Dd[64:96], Dt[64:96])
                nc.gpsimd.dma_start(Dd[96:128], Dt[96:128])
                um = p_small.tile([128, NT], dt.bfloat16, tag="um")
                nc.vector.tensor_scalar(um[:], st["rt"][:], FIRE, None,
                                        op.is_lt)
                st.update(Dd=Dd, um=um)

            def emit_head2(s, st):
                alP = p_small.tile([128, NT], dt.bfloat16, tag="alP")
                nc.vector.tensor_copy(alP[:], st["xt3"][:, :, 3])
                preM = p_small.tile([128, NT], dt.bfloat16, tag="preM")
                _pool_and_thresh(nc, p_pscr, alP, preM, op, dt)
                st["preM"] = preM

            def emit_mid(s, st, strips, counters):
                xt, um = st["xt"], st["um"]
                Vd, Dd = st["Vd"], st["Dd"]
                F = SOB_F
                for hb in strips:
                    S = p_S.tile([KROWS, SOB_F], sdt, tag="S")
                    pp = slice(16 * hb, 16 * hb + 16)
                    # contiguous DRAM->SBUF shift gathers
                    # all gathers on the sync (HWDGE) ring -- the gpsimd
                    # ring is busy with the casting V/D stores, and a
                    # gather queued behind them stalls the first matmuls
                    nc.sync.dma_start(S[0:16, :],
                                      xg_d.ap()[s, pp, PITCH:PITCH + F])
                    nc.sync.dma_start(S[16:32, 1:F], Vd[pp, 0:F - 1])
                    nc.sync.dma_start(S[32:48, 0:F - 1], Vd[pp, 1:F])
                    nc.sync.dma_start(S[48:64, 1:F], Dd[pp, 0:F - 1])
                    nc.sync.dma_start(S[64:80, :], Dd[pp, :])
                    nc.sync.dma_start(S[80:96, 0:F - 1], Dd[pp, 1:F])

                    hsb = p_hsb.tile([128, SOB_F], dt.bfloat16, tag="hsb")
                    psdx = p_psdx.tile([128, 1024], dt.float32, tag="psdx")

                    def emit_mm2(limit):
                        # mm2 for pixel tiles whose hsb window is fully
                        # evacuated (off+128 <= limit)
                        while True:
                            t = counters["t_next"]
                            if t >= 64:
                                break
                            off = (t // 2) * PITCH + 1 + (t % 2) * 128
                            if off + 128 > limit:
                                break
                            nc.tensor.matmul(
                                psdx[:, 16 * t:16 * t + 16],
                                hsb[:, off:off + 128],
                                w2_sb[:])
                            counters["t_next"] += 1

                    # software-pipelined by two chunks: mm2 of chunk i-2
                    # is emitted after mm1 of chunk i, so the PE never
                    # waits in-order on a just-triggered evac.
                    ends = [0, 0]
                    for (c0, cw) in CHUNKS:
                        psh = p_psh.tile([128, 1536], dt.float32, tag="psh")
                        nmm = (cw + 511) // 512
                        for j in range(nmm):
                            w = min(512, cw - 512 * j)
                            nc.tensor.matmul(
                                psh[:, 512 * j:512 * j + w],
                                w1_sb[:],
                                S[:, c0 + 512 * j:c0 + 512 * j + w])
                        emit_mm2(ends[-2])
                        ends.append(c0 + cw)
                        ci = counters["chunk"]
                        counters["chunk"] += 1
                        if ci % EVAC_V_EVERY == EVAC_V_EVERY - 1:
                            if S_FP8:
                                nc.vector.tensor_scalar(
                                    hsb[:, c0:c0 + cw], psh[:, :cw],
                                    0.125, 0.0, op.mult, op.max)
                            else:
                                nc.vector.tensor_scalar(
                                    hsb[:, c0:c0 + cw], psh[:, :cw],
                                    0.0, None, op.max)
                        else:
                            nc.scalar.activation(
                                hsb[:, c0:c0 + cw], psh[:, :cw], AF.Relu,
                                scale=0.125 if S_FP8 else 1.0)
                    emit_mm2(ends[-2])
                    emit_mm2(SOB_F)
                    counters["t_next"] = 0
                    _evac_strip(nc, psdx, hb, um, xt, op, dt, p_dx,
                                b2_sb if b2_nonzero else None)

            def emit_tail(s, st):
                xt, xt3 = st["xt"], st["xt3"]
                alN = p_small.tile([128, NT], dt.bfloat16, tag="alN")
                nc.vector.tensor_copy(alN[:], xt3[:, :, 3])
                postM = p_small.tile([128, NT], dt.bfloat16, tag="postM")
                _pool_and_thresh(nc, p_pscr, alN, postM, op, dt)
                life = p_small.tile([128, NT], dt.bfloat16, tag="life")
                nc.vector.tensor_tensor(life[:], st["preM"][:], postM[:],
                                        op.mult)
                nh = len(MULT_PAT)
                tq = NT // nh
                for q in range(nh):
                    ts = slice(tq * q, tq * (q + 1))
                    cs = slice(16 * tq * q, 16 * tq * (q + 1))
                    eng(MULT_PAT[q]).tensor_tensor(
                        xt3[:, ts, :], xt3[:, ts, :],
                        life[:, ts].broadcast_to([128, tq, 16]), op.mult)
                    nc.gpsimd.dma_start(out_d.ap()[s][:, cs], xt[:, cs])

            counters = {"chunk": 0, "t_next": 0}
            st0 = emit_head_loads(0)
            emit_sobel_a(0, st0)
            emit_sobel_v(0, st0)
            emit_sobel_d(0, st0)
            emit_head2(0, st0)
            emit_mid(0, st0, range(0, 3), counters)
            st1 = emit_head_loads(1)
            emit_sobel_a(1, st1)
            emit_mid(0, st0, range(3, 4), counters)
            emit_sobel_v(1, st1)
            emit_mid(0, st0, range(4, 5), counters)
            emit_sobel_d(1, st1)
            emit_mid(0, st0, range(5, 8), counters)
            emit_head2(1, st1)
            emit_tail(0, st0)
            emit_mid(1, st1, range(0, 8), counters)
            emit_tail(1, st1)

    nc.compile()
    return nc


def _evac_strip(nc, psdx, hb, um, xt, op, dt, p_dx, b2_sb):
    """Strip hb (8192 px, 64 tiles): dx*um and x += in pixel-major."""
    ps3 = psdx.rearrange("p (t c) -> p t c", c=16)
    umk = um[:, 64 * hb:64 * hb + 64]
    sl = slice(1024 * hb, 1024 * (hb + 1))
    if b2_sb is not None:
        nc.vector.tensor_tensor(
            ps3[:], ps3[:],
            b2_sb[:].rearrange("p c -> p 1 c").broadcast_to([128, 64, 16]),
            op.add)
    DXM = p_dx.tile([128, 1024], dt.bfloat16, tag="DXM")
    nc.vector.tensor_tensor(
        DXM.rearrange("p (t c) -> p t c", c=16), ps3[:],
        umk.broadcast_to([128, 64, 16]), op.mult)
    nc.vector.tensor_tensor(xt[:, sl], xt[:, sl], DXM[:], op.add)


def _pool_and_thresh(nc, pool, alpha, outM, op, dt):
    """3x3 circular max-pool on pixel-major alpha [128, NT] then > ALPHA_TH.

    Neighbor tensors staged entirely by DMA, then one straight-line
    Vector block.
    """
    f16 = dt.bfloat16
    aL = pool.tile([128, NT], f16, tag="aL")
    aR = pool.tile([128, NT], f16, tag="aR")
    nc.sync.dma_start(aL[1:128, :], alpha[0:127, :])
    nc.gpsimd.dma_start(aR[0:127, :], alpha[1:128, :])
    nc.sync.dma_start(aL[0:1, 0:NT:2], alpha[127:128, 1:NT:2])
    nc.sync.dma_start(aL[0:1, 1:NT:2], alpha[127:128, 0:NT - 1:2])
    nc.gpsimd.dma_start(aR[127:128, 0:NT:2], alpha[0:1, 1:NT:2])
    nc.gpsimd.dma_start(aR[127:128, 1:NT:2], alpha[0:1, 0:NT - 1:2])
    PW = pool.tile([128, NT], f16, tag="PW")
    nc.vector.tensor_tensor(PW[:], alpha[:, :], aL[:], op.max)
    nc.vector.tensor_tensor(PW[:], PW[:], aR[:], op.max)
    z2 = pool.tile([128, NT], f16, tag="z2")
    nc.vector.tensor_tensor(z2[:, 0:NT - 2], PW[:, 0:NT - 2], PW[:, 2:NT], op.max)
    nc.vector.tensor_tensor(outM[:, 2:NT - 2], z2[:, 0:NT - 4], PW[:, 4:NT], op.max)
    nc.vector.tensor_tensor(outM[:, 0:2], z2[:, 0:2], PW[:, NT - 2:NT], op.max)
    nc.vector.tensor_tensor(outM[:, NT - 2:NT], z2[:, NT - 4:NT - 2],
                            PW[:, 0:2], op.max)
    nc.vector.tensor_scalar(outM[:], outM[:], ALPHA_TH, None, op.is_gt)


def _get_built(b2_nonzero):
    global _BUILT
    if _BUILT is None or _BUILT[0] != b2_nonzero:
        _BUILT = (b2_nonzero, _build(b2_nonzero))
    return _BUILT[1]


# ------------------------------------------------------------------ kernel
def kernel(x, rand_vals, w1, b1, w2, b2):
    from concourse.bass_utils import run_bass_kernel_spmd

    x = np.asarray(x, np.float32)
    rand_vals = np.asarray(rand_vals, np.float32)
    w1e, w2e, b2e = _prep_weights(w1, b1, w2, b2)
    b2_nonzero = bool(np.any(b2e != 0.0))

    xbf = _prep_xbf(x, _bf16())
    xt = _prep_xt(x)
    rt = _prep_randt(rand_vals)
    sdt_np = _fp8() if S_FP8 else _bf16()
    ones = np.ones((1, SOB_F), dtype=sdt_np)

    nc = _get_built(b2_nonzero)

    in_maps = []
    for i in range(NCORES):
        sl = slice(SPC * i, SPC * (i + 1))
        m = {
            "xbf": np.ascontiguousarray(xbf[sl]),
            "xt": np.ascontiguousarray(xt[sl]),
            "rt": np.ascontiguousarray(rt[sl]),
            "w1e": w1e, "w2e": w2e, "onesr": ones,
            "b2e": b2e.reshape(1, 16),
        }
        if S_FP8:
            m["xf8"] = np.ascontiguousarray(
                _prep_xbf(x, _fp8())[sl])
        in_maps.append(m)

    res = run_bass_kernel_spmd(nc, in_maps, core_ids=list(range(NCORES)))
    outs = [res.results[i]["outp"] for i in range(NCORES)]
    out_pm = np.concatenate(outs, axis=0)
    return _unprep_out(out_pm)


# revision 55
# speedup vs baseline: 1.0384x; 1.0384x over previous
"""Trainium2 Bass kernel for nn_CAModel (neural cellular automaton step).

v4 strategy (data-parallel over batch, 16 samples -> 8 cores x 2):
  - w-direction sobel taps folded into mm1's contraction dim (K=97):
    S rows = [x(16); V(w-1); V(w+1); D(w-1); D(w); D(w+1); ones], with
    V = [1,2,1]_h smoothing, D = x(h+1)-x(h-1), ones carrying b1.
  - S staging lives in padded pixel space (pitch 258): all shift gathers
    are contiguous; pixel tiles (128) never straddle a row.
  - 16-consecutive-partition SBUF->SBUF DMAs only reach 4/16 SDMA ports,
    so V/D round-trip through DRAM: one wide store each, then fast
    DRAM->SBUF block gathers (x rows gather straight from the xbf DRAM
    copy).  DRAM scratch comes from a DRAM tile pool so store->gather
    RAW hazards are tracked.
  - relu evac = max(psum,0) split between ScalarE ACTIVATE and VectorE
    TENSOR_SCALAR; the Scalar ring issues no DMAs at all.
  - x bf16 end-to-end; per-strip hsb/psdx; bank evac + masks on Vector;
    pool neighbor staging entirely by DMA.
Host does layout transforms only; HW exec time is what's measured.
"""

import numpy as np

# ---------------------------------------------------------------- constants
B, C, H, W = 16, 16, 256, 256
NCORES = 8
SPC = B // NCORES
HWPX = H * W
PITCH = 258
NROWH = 34
XBF_F = NROWH * PITCH      # 8772
SOB_F = 32 * PITCH         # 8256
PIX_F = 8192
NT = HWPX // 128           # 512
NSTRIP = 8
KROWS = 97
ALPHA_TH = 0.1
FIRE = 0.5

CHUNKS = [(0, 1536), (1536, 1536), (3072, 1536), (4608, 1536),
          (6144, 1536), (7680, 576)]

# ------------------------------------------------------------ tuning knobs
S_FP8 = True          # fp8e4m3 S staging (w1e scaled x8, evac scales 1/8)
EVAC_V_EVERY = 4      # every Nth chunk's relu evac goes to Vector (rest S)
MULT_PAT = "VV"       # final x*life halves (GpSimd TT is ~2.5x slower)

_BUILT = None


# ------------------------------------------------------------- host layouts
def _bf16():
    import ml_dtypes
    return ml_dtypes.bfloat16


def _fp8():
    import ml_dtypes
    return ml_dtypes.float8_e4m3fn


def _pad_wrap(a):
    out = np.empty(a.shape[:-1] + (PITCH,), dtype=a.dtype)
    out[..., 1:257] = a
    out[..., 0] = a[..., 255]
    out[..., 257] = a[..., 0]
    return out


def _strip_rows(x):
    """x [B,C,H,W] -> [B, 8, C, 34, 258] with halo rows and wrap cols."""
    hidx = (np.arange(-1, 33)[None, :] + 32 * np.arange(8)[:, None]) % 256
    xr = x[:, :, hidx, :]                                  # [B, C, 8, 34, W]
    return _pad_wrap(np.transpose(xr, (0, 2, 1, 3, 4)))


def _prep_xbf(x, dtype):
    out = _strip_rows(x).astype(dtype)
    return np.ascontiguousarray(out.reshape(B, 128, XBF_F))


def _prep_xt(x):
    bf16 = _bf16()
    xf = x.reshape(B, C, HWPX).transpose(0, 2, 1)
    xf = xf.reshape(B, NT, 128, C).transpose(0, 2, 1, 3)
    return np.ascontiguousarray(xf.reshape(B, 128, NT * C).astype(bf16))


def _prep_randt(rv):
    rf = rv.reshape(B, HWPX).reshape(B, NT, 128).transpose(0, 2, 1)
    return np.ascontiguousarray(rf.astype(np.float32))


def _unprep_out(op):
    o = op.astype(np.float32).reshape(B, 128, NT, C).transpose(0, 2, 1, 3)
    o = o.reshape(B, HWPX, C).transpose(0, 2, 1)
    return np.ascontiguousarray(o.reshape(B, C, H, W))


def _prep_weights(w1, b1, w2, b2):
    bf16 = _bf16()
    w1 = np.asarray(w1, np.float32)
    w2 = np.asarray(w2, np.float32)
    wid, wdx, wdy = w1[0::3], w1[1::3], w1[2::3]
    w1e = np.concatenate([
        wid,
        -0.125 * wdx,          # V(w-1)
        0.125 * wdx,           # V(w+1)
        0.125 * wdy,           # D(w-1)
        0.25 * wdy,            # D(w)
        0.125 * wdy,           # D(w+1)
        np.asarray(b1, np.float32).reshape(1, 128),
    ], axis=0)                                            # [97, 128]
    if S_FP8:
        w1e = np.ascontiguousarray((8.0 * w1e).astype(_fp8()))
    else:
        w1e = np.ascontiguousarray(w1e.astype(bf16))
    return (w1e,
            np.ascontiguousarray(w2.astype(bf16)),
            np.asarray(b2, np.float32).reshape(1, 16))


# ------------------------------------------------------------- build module
def _build(b2_nonzero):
    import concourse.bass as bass
    import concourse.bacc as bacc
    import concourse.mybir as mybir
    import concourse.tile as tile

    dt = mybir.dt
    op = mybir.AluOpType
    AF = mybir.ActivationFunctionType
    sdt = dt.float8e4 if S_FP8 else dt.bfloat16

    nc = bacc.Bacc("TRN2", target_bir_lowering=False, debug=False)

    xbf_d = nc.dram_tensor("xbf", (SPC, 128, XBF_F), dt.bfloat16, kind="ExternalInput")
    xg_d = (nc.dram_tensor("xf8", (SPC, 128, XBF_F), sdt, kind="ExternalInput")
            if S_FP8 else xbf_d)
    xt_d = nc.dram_tensor("xt", (SPC, 128, PIX_F), dt.bfloat16, kind="ExternalInput")
    rt_d = nc.dram_tensor("rt", (SPC, 128, NT), dt.float32, kind="ExternalInput")
    w1_d = nc.dram_tensor("w1e", (KROWS, 128), sdt, kind="ExternalInput")
    w2_d = nc.dram_tensor("w2e", (128, 16), dt.bfloat16, kind="ExternalInput")
    ones_d = nc.dram_tensor("onesr", (1, SOB_F), sdt, kind="ExternalInput")
    b2_d = nc.dram_tensor("b2e", (1, 16), dt.float32, kind="ExternalInput")
    out_d = nc.dram_tensor("outp", (SPC, 128, PIX_F), dt.bfloat16, kind="ExternalOutput")

    def eng(name):
        return {"V": nc.vector, "P": nc.gpsimd}[name]

    with tile.TileContext(nc) as tc:
        with (
            tc.tile_pool(name="wpool", bufs=1) as wpool,
            tc.tile_pool(name="xbf", bufs=1) as p_xbf,
            tc.tile_pool(name="pA", bufs=1) as p_A,
            tc.tile_pool(name="pV", bufs=1) as p_V,
            tc.tile_pool(name="pD", bufs=1) as p_D,
            tc.tile_pool(name="xt", bufs=2) as p_xt,
            tc.tile_pool(name="S", bufs=3) as p_S,
            tc.tile_pool(name="hsb", bufs=2) as p_hsb,
            tc.tile_pool(name="small", bufs=2) as p_small,
            tc.tile_pool(name="dx", bufs=2) as p_dx,
            tc.tile_pool(name="pscr", bufs=2) as p_pscr,
            tc.tile_pool(name="vdd", bufs=2, space="DRAM") as p_vdd,
            tc.tile_pool(name="psh", bufs=2, space=bass.MemorySpace.PSUM) as p_psh,
            tc.tile_pool(name="psdx", bufs=1, space=bass.MemorySpace.PSUM) as p_psdx,
        ):
            w1_sb = wpool.tile([KROWS, 128], sdt, tag="w1")
            nc.sync.dma_start(w1_sb[:], w1_d.ap())
            w2_sb = wpool.tile([128, 16], dt.bfloat16, tag="w2")
            nc.sync.dma_start(w2_sb[:], w2_d.ap())
            if b2_nonzero:
                b2_sb = wpool.tile([128, 16], dt.float32, tag="b2")
                nc.sync.dma_start(b2_sb[:], b2_d.ap().broadcast_to([128, 16]))

            # prime all S slots: ones row + edge cols the contiguous
            # shift gathers never write (all persist across slot reuse).
            for _ in range(3):
                St = p_S.tile([KROWS, SOB_F], sdt, tag="S")
                nc.sync.dma_start(St[96:97, :], ones_d.ap())
                nc.vector.memset(St[:, 0:1], 0.0)
                nc.vector.memset(St[:, SOB_F - 1:SOB_F], 0.0)

            def emit_head_loads(s):
                st = {}
                xbf = p_xbf.tile([128, XBF_F], dt.bfloat16, tag="xbf")
                nc.sync.dma_start(xbf[0:64, :], xbf_d.ap()[s, 0:64])
                nc.gpsimd.dma_start(xbf[64:128, :], xbf_d.ap()[s, 64:128])
                xt = p_xt.tile([128, PIX_F], dt.bfloat16, tag="xt")
                nc.gpsimd.dma_start(xt[:], xt_d.ap()[s])
                rt = p_small.tile([128, NT], dt.float32, tag="rt")
                nc.gpsimd.dma_start(rt[:], rt_d.ap()[s])
                xbf3 = xbf.rearrange("p (r q) -> p r q", q=PITCH)
                st.update(xbf3=xbf3, xt=xt, rt=rt,
                          xt3=xt.rearrange("p (t c) -> p t c", c=16))
                return st

            def emit_sobel_a(s, st):
                A = p_A.tile([128, SOB_F], dt.bfloat16, tag="A")
                nc.vector.tensor_tensor(
                    A.rearrange("p (r q) -> p r q", q=PITCH)[:],
                    st["xbf3"][:, 0:32, :], st["xbf3"][:, 2:34, :], op.add)
                st["A"] = A

            def emit_sobel_v(s, st):
                Vt = p_V.tile([128, SOB_F], dt.bfloat16, tag="V")
                nc.vector.scalar_tensor_tensor(
                    Vt.rearrange("p (r q) -> p r q", q=PITCH)[:],
                    st["xbf3"][:, 1:33, :], 2.0,
                    st["A"].rearrange("p (r q) -> p r q", q=PITCH)[:],
                    op.mult, op.add)
                # quarter-stores: strip hb's gather only reads partitions
                # 16hb..16hb+16, so early strips unblock after 1/4 of the
                # (slow, casting) store instead of all of it
                Vd = p_vdd.tile([128, SOB_F], sdt, tag="Vd")
                nc.gpsimd.dma_start(Vd[0:32], Vt[0:32])
                nc.gpsimd.dma_start(Vd[32:64], Vt[32:64])
                st.update(Vd=Vd, Vt=Vt)

            def emit_sobel_d(s, st):
                Dt = p_D.tile([128, SOB_F], dt.bfloat16, tag="D")
                nc.vector.tensor_tensor(
                    Dt.rearrange("p (r q) -> p r q", q=PITCH)[:],
                    st["xbf3"][:, 2:34, :], st["xbf3"][:, 0:32, :],
                    op.subtract)
                Dd = p_vdd.tile([128, SOB_F], sdt, tag="Dd")
                Vd, Vt = st["Vd"], st["Vt"]
                # interleave remaining V and D quarter-stores so strip 0
                # (needs V[0:16]+D[0:16]) unblocks first
                nc.gpsimd.dma_start(Dd[0:32], Dt[0:32])
                nc.gpsimd.dma_start(Vd[64:96], Vt[64:96])
                nc.gpsimd.dma_start(Dd[32:64], Dt[32:64])
                nc.gpsimd.dma_start(Vd[96:128], Vt[96:128])
                nc.gpsimd.dma_start(Dd[64:96], Dt[64:96])
                nc.gpsimd.dma_start(Dd[96:128], Dt[96:128])
                um = p_small.tile([128, NT], dt.bfloat16, tag="um")
                nc.vector.tensor_scalar(um[:], st["rt"][:], FIRE, None,
                                        op.is_lt)
                st.update(Dd=Dd, um=um)

            def emit_head2(s, st):
                alP = p_small.tile([128, NT], dt.bfloat16, tag="alP")
                nc.vector.tensor_copy(alP[:], st["xt3"][:, :, 3])
                preM = p_small.tile([128, NT], dt.bfloat16, tag="preM")
                _pool_and_thresh(nc, p_pscr, alP, preM, op, dt)
                st["preM"] = preM

            def emit_mid(s, st, strips, counters, d_on_sync=False):
                xt, um = st["xt"], st["um"]
                Vd, Dd = st["Vd"], st["Dd"]
                F = SOB_F
                dq = nc.sync if d_on_sync else nc.gpsimd
                for hb in strips:
                    S = p_S.tile([KROWS, SOB_F], sdt, tag="S")
                    pp = slice(16 * hb, 16 * hb + 16)
                    # contiguous DRAM->SBUF shift gathers
                    nc.sync.dma_start(S[0:16, :],
                                      xg_d.ap()[s, pp, PITCH:PITCH + F])
                    nc.sync.dma_start(S[16:32, 1:F], Vd[pp, 0:F - 1])
                    nc.sync.dma_start(S[32:48, 0:F - 1], Vd[pp, 1:F])
                    dq.dma_start(S[48:64, 1:F], Dd[pp, 0:F - 1])
                    nc.sync.dma_start(S[64:80, :], Dd[pp, :])
                    dq.dma_start(S[80:96, 0:F - 1], Dd[pp, 1:F])

                    hsb = p_hsb.tile([128, SOB_F], dt.bfloat16, tag="hsb")
                    psdx = p_psdx.tile([128, 1024], dt.float32, tag="psdx")

                    def emit_mm2(limit):
                        # mm2 for pixel tiles whose hsb window is fully
                        # evacuated (off+128 <= limit)
                        while True:
                            t = counters["t_next"]
                            if t >= 64:
                                break
                            off = (t // 2) * PITCH + 1 + (t % 2) * 128
                            if off + 128 > limit:
                                break
                            nc.tensor.matmul(
                                psdx[:, 16 * t:16 * t + 16],
                                hsb[:, off:off + 128],
                                w2_sb[:])
                            counters["t_next"] += 1

                    # software-pipelined by two chunks: mm2 of chunk i-2
                    # is emitted after mm1 of chunk i, so the PE never
                    # waits in-order on a just-triggered evac.
                    ends = [0, 0]
                    for (c0, cw) in CHUNKS:
                        psh = p_psh.tile([128, 1536], dt.float32, tag="psh")
                        nmm = (cw + 511) // 512
                        for j in range(nmm):
                            w = min(512, cw - 512 * j)
                            nc.tensor.matmul(
                                psh[:, 512 * j:512 * j + w],
                                w1_sb[:],
                                S[:, c0 + 512 * j:c0 + 512 * j + w])
                        emit_mm2(ends[-2])
                        ends.append(c0 + cw)
                        ci = counters["chunk"]
                        counters["chunk"] += 1
                        if ci % EVAC_V_EVERY == EVAC_V_EVERY - 1:
                            if S_FP8:
                                nc.vector.tensor_scalar(
                                    hsb[:, c0:c0 + cw], psh[:, :cw],
                                    0.125, 0.0, op.mult, op.max)
                            else:
                                nc.vector.tensor_scalar(
                                    hsb[:, c0:c0 + cw], psh[:, :cw],
                                    0.0, None, op.max)
                        else:
                            nc.scalar.activation(
                                hsb[:, c0:c0 + cw], psh[:, :cw], AF.Relu,
                                scale=0.125 if S_FP8 else 1.0)
                    emit_mm2(ends[-2])
                    emit_mm2(SOB_F)
                    counters["t_next"] = 0
                    _evac_strip(nc, psdx, hb, um, xt, op, dt, p_dx,
                                b2_sb if b2_nonzero else None)

            def emit_tail(s, st):
                xt, xt3 = st["xt"], st["xt3"]
                alN = p_small.tile([128, NT], dt.bfloat16, tag="alN")
                nc.vector.tensor_copy(alN[:], xt3[:, :, 3])
                postM = p_small.tile([128, NT], dt.bfloat16, tag="postM")
                _pool_and_thresh(nc, p_pscr, alN, postM, op, dt)
                life = p_small.tile([128, NT], dt.bfloat16, tag="life")
                nc.vector.tensor_tensor(life[:], st["preM"][:], postM[:],
                                        op.mult)
                nh = len(MULT_PAT)
                tq = NT // nh
                for q in range(nh):
                    ts = slice(tq * q, tq * (q + 1))
                    cs = slice(16 * tq * q, 16 * tq * (q + 1))
                    eng(MULT_PAT[q]).tensor_tensor(
                        xt3[:, ts, :], xt3[:, ts, :],
                        life[:, ts].broadcast_to([128, tq, 16]), op.mult)
                    nc.gpsimd.dma_start(out_d.ap()[s][:, cs], xt[:, cs])

            counters = {"chunk": 0, "t_next": 0}
            st0 = emit_head_loads(0)
            emit_sobel_a(0, st0)
            emit_sobel_v(0, st0)
            emit_sobel_d(0, st0)
            emit_head2(0, st0)
            # head window: sync ring is otherwise idle, and the gpsimd
            # ring is draining the casting V/D stores -- route the first
            # two strips' D gathers around that queue
            emit_mid(0, st0, range(0, 2), counters, d_on_sync=True)
            emit_mid(0, st0, range(2, 3), counters)
            st1 = emit_head_loads(1)
            emit_sobel_a(1, st1)
            emit_mid(0, st0, range(3, 4), counters)
            emit_sobel_v(1, st1)
            emit_mid(0, st0, range(4, 5), counters)
            emit_sobel_d(1, st1)
            emit_mid(0, st0, range(5, 8), counters)
            emit_head2(1, st1)
            emit_tail(0, st0)
            emit_mid(1, st1, range(0, 8), counters)
            emit_tail(1, st1)

    nc.compile()
    return nc


def _evac_strip(nc, psdx, hb, um, xt, op, dt, p_dx, b2_sb):
    """Strip hb (8192 px, 64 tiles): dx*um and x += in pixel-major."""
    ps3 = psdx.rearrange("p (t c) -> p t c", c=16)
    umk = um[:, 64 * hb:64 * hb + 64]
    sl = slice(1024 * hb, 1024 * (hb + 1))
    if b2_sb is not None:
        nc.vector.tensor_tensor(
            ps3[:], ps3[:],
            b2_sb[:].rearrange("p c -> p 1 c").broadcast_to([128, 64, 16]),
            op.add)
    DXM = p_dx.tile([128, 1024], dt.bfloat16, tag="DXM")
    nc.vector.tensor_tensor(
        DXM.rearrange("p (t c) -> p t c", c=16), ps3[:],
        umk.broadcast_to([128, 64, 16]), op.mult)
    nc.vector.tensor_tensor(xt[:, sl], xt[:, sl], DXM[:], op.add)


def _pool_and_thresh(nc, pool, alpha, outM, op, dt):
    """3x3 circular max-pool on pixel-major alpha [128, NT] then > ALPHA_TH.

    Neighbor tensors staged entirely by DMA, then one straight-line
    Vector block.
    """
    f16 = dt.bfloat16
    aL = pool.tile([128, NT], f16, tag="aL")
    aR = pool.tile([128, NT], f16, tag="aR")
    nc.sync.dma_start(aL[1:128, :], alpha[0:127, :])
    nc.gpsimd.dma_start(aR[0:127, :], alpha[1:128, :])
    nc.sync.dma_start(aL[0:1, 0:NT:2], alpha[127:128, 1:NT:2])
    nc.sync.dma_start(aL[0:1, 1:NT:2], alpha[127:128, 0:NT - 1:2])
    nc.gpsimd.dma_start(aR[127:128, 0:NT:2], alpha[0:1, 1:NT:2])
    nc.gpsimd.dma_start(aR[127:128, 1:NT:2], alpha[0:1, 0:NT - 1:2])
    PW = pool.tile([128, NT], f16, tag="PW")
    nc.vector.tensor_tensor(PW[:], alpha[:, :], aL[:], op.max)
    nc.vector.tensor_tensor(PW[:], PW[:], aR[:], op.max)
    z2 = pool.tile([128, NT], f16, tag="z2")
    nc.vector.tensor_tensor(z2[:, 0:NT - 2], PW[:, 0:NT - 2], PW[:, 2:NT], op.max)
    nc.vector.tensor_tensor(outM[:, 2:NT - 2], z2[:, 0:NT - 4], PW[:, 4:NT], op.max)
    nc.vector.tensor_tensor(outM[:, 0:2], z2[:, 0:2], PW[:, NT - 2:NT], op.max)
    nc.vector.tensor_tensor(outM[:, NT - 2:NT], z2[:, NT - 4:NT - 2],
                            PW[:, 0:2], op.max)
    nc.vector.tensor_scalar(outM[:], outM[:], ALPHA_TH, None, op.is_gt)


def _get_built(b2_nonzero):
    global _BUILT
    if _BUILT is None or _BUILT[0] != b2_nonzero:
        _BUILT = (b2_nonzero, _build(b2_nonzero))
    return _BUILT[1]


# ------------------------------------------------------------------ kernel
def kernel(x, rand_vals, w1, b1, w2, b2):
    from concourse.bass_utils import run_bass_kernel_spmd

    x = np.asarray(x, np.float32)
    rand_vals = np.asarray(rand_vals, np.float32)
    w1e, w2e, b2e = _prep_weights(w1, b1, w2, b2)
    b2_nonzero = bool(np.any(b2e != 0.0))

    xbf = _prep_xbf(x, _bf16())
    xt = _prep_xt(x)
    rt = _prep_randt(rand_vals)
    sdt_np = _fp8() if S_FP8 else _bf16()
    ones = np.ones((1, SOB_F), dtype=sdt_np)

    nc = _get_built(b2_nonzero)

    in_maps = []
    for i in range(NCORES):
        sl = slice(SPC * i, SPC * (i + 1))
        m = {
            "xbf": np.ascontiguousarray(xbf[sl]),
            "xt": np.ascontiguousarray(xt[sl]),
            "rt": np.ascontiguousarray(rt[sl]),
            "w1e": w1e, "w2e": w2e, "onesr": ones,
            "b2e": b2e.reshape(1, 16),
        }
        if S_FP8:
            m["xf8"] = np.ascontiguousarray(
                _prep_xbf(x, _fp8())[sl])
        in_maps.append(m)

    res = run_bass_kernel_spmd(nc, in_maps, core_ids=list(range(NCORES)))
    outs = [res.results[i]["outp"] for i in range(NCORES)]
    out_pm = np.concatenate(outs, axis=0)
    return _unprep_out(out_pm)


# revision 58
# speedup vs baseline: 1.0420x; 1.0035x over previous
"""Trainium2 Bass kernel for nn_CAModel (neural cellular automaton step).

v4 strategy (data-parallel over batch, 16 samples -> 8 cores x 2):
  - w-direction sobel taps folded into mm1's contraction dim (K=97):
    S rows = [x(16); V(w-1); V(w+1); D(w-1); D(w); D(w+1); ones], with
    V = [1,2,1]_h smoothing, D = x(h+1)-x(h-1), ones carrying b1.
  - S staging lives in padded pixel space (pitch 258): all shift gathers
    are contiguous; pixel tiles (128) never straddle a row.
  - 16-consecutive-partition SBUF->SBUF DMAs only reach 4/16 SDMA ports,
    so V/D round-trip through DRAM: one wide store each, then fast
    DRAM->SBUF block gathers (x rows gather straight from the xbf DRAM
    copy).  DRAM scratch comes from a DRAM tile pool so store->gather
    RAW hazards are tracked.
  - relu evac = max(psum,0) split between ScalarE ACTIVATE and VectorE
    TENSOR_SCALAR; the Scalar ring issues no DMAs at all.
  - x bf16 end-to-end; per-strip hsb/psdx; bank evac + masks on Vector;
    pool neighbor staging entirely by DMA.
Host does layout transforms only; HW exec time is what's measured.
"""

import numpy as np

# ---------------------------------------------------------------- constants
B, C, H, W = 16, 16, 256, 256
NCORES = 8
SPC = B // NCORES
HWPX = H * W
PITCH = 258
NROWH = 34
XBF_F = NROWH * PITCH      # 8772
SOB_F = 32 * PITCH         # 8256
PIX_F = 8192
NT = HWPX // 128           # 512
NSTRIP = 8
KROWS = 97
ALPHA_TH = 0.1
FIRE = 0.5

CHUNKS = [(0, 1536), (1536, 1536), (3072, 1536), (4608, 1536),
          (6144, 1536), (7680, 576)]

# ------------------------------------------------------------ tuning knobs
S_FP8 = True          # fp8e4m3 S staging (w1e scaled x8, evac scales 1/8)
EVAC_V_EVERY = 4      # every Nth chunk's relu evac goes to Vector (rest S)
MULT_PAT = "VV"       # final x*life halves (GpSimd TT is ~2.5x slower)

_BUILT = None


# ------------------------------------------------------------- host layouts
def _bf16():
    import ml_dtypes
    return ml_dtypes.bfloat16


def _fp8():
    import ml_dtypes
    return ml_dtypes.float8_e4m3fn


def _pad_wrap(a):
    out = np.empty(a.shape[:-1] + (PITCH,), dtype=a.dtype)
    out[..., 1:257] = a
    out[..., 0] = a[..., 255]
    out[..., 257] = a[..., 0]
    return out


def _strip_rows(x):
    """x [B,C,H,W] -> [B, 8, C, 34, 258] with halo rows and wrap cols."""
    hidx = (np.arange(-1, 33)[None, :] + 32 * np.arange(8)[:, None]) % 256
    xr = x[:, :, hidx, :]                                  # [B, C, 8, 34, W]
    return _pad_wrap(np.transpose(xr, (0, 2, 1, 3, 4)))


def _prep_xbf(x, dtype):
    out = _strip_rows(x).astype(dtype)
    return np.ascontiguousarray(out.reshape(B, 128, XBF_F))


def _prep_xt(x):
    bf16 = _bf16()
    xf = x.reshape(B, C, HWPX).transpose(0, 2, 1)
    xf = xf.reshape(B, NT, 128, C).transpose(0, 2, 1, 3)
    return np.ascontiguousarray(xf.reshape(B, 128, NT * C).astype(bf16))


def _prep_randt(rv):
    rf = rv.reshape(B, HWPX).reshape(B, NT, 128).transpose(0, 2, 1)
    return np.ascontiguousarray(rf.astype(np.float32))


def _unprep_out(op):
    o = op.astype(np.float32).reshape(B, 128, NT, C).transpose(0, 2, 1, 3)
    o = o.reshape(B, HWPX, C).transpose(0, 2, 1)
    return np.ascontiguousarray(o.reshape(B, C, H, W))


def _prep_weights(w1, b1, w2, b2):
    bf16 = _bf16()
    w1 = np.asarray(w1, np.float32)
    w2 = np.asarray(w2, np.float32)
    wid, wdx, wdy = w1[0::3], w1[1::3], w1[2::3]
    w1e = np.concatenate([
        wid,
        -0.125 * wdx,          # V(w-1)
        0.125 * wdx,           # V(w+1)
        0.125 * wdy,           # D(w-1)
        0.25 * wdy,            # D(w)
        0.125 * wdy,           # D(w+1)
        np.asarray(b1, np.float32).reshape(1, 128),
    ], axis=0)                                            # [97, 128]
    if S_FP8:
        w1e = np.ascontiguousarray((8.0 * w1e).astype(_fp8()))
    else:
        w1e = np.ascontiguousarray(w1e.astype(bf16))
    return (w1e,
            np.ascontiguousarray(w2.astype(bf16)),
            np.asarray(b2, np.float32).reshape(1, 16))


# ------------------------------------------------------------- build module
def _build(b2_nonzero):
    import concourse.bass as bass
    import concourse.bacc as bacc
    import concourse.mybir as mybir
    import concourse.tile as tile

    dt = mybir.dt
    op = mybir.AluOpType
    AF = mybir.ActivationFunctionType
    sdt = dt.float8e4 if S_FP8 else dt.bfloat16

    nc = bacc.Bacc("TRN2", target_bir_lowering=False, debug=False)

    xbf_d = nc.dram_tensor("xbf", (SPC, 128, XBF_F), dt.bfloat16, kind="ExternalInput")
    xg_d = (nc.dram_tensor("xf8", (SPC, 128, XBF_F), sdt, kind="ExternalInput")
            if S_FP8 else xbf_d)
    xt_d = nc.dram_tensor("xt", (SPC, 128, PIX_F), dt.bfloat16, kind="ExternalInput")
    rt_d = nc.dram_tensor("rt", (SPC, 128, NT), dt.float32, kind="ExternalInput")
    w1_d = nc.dram_tensor("w1e", (KROWS, 128), sdt, kind="ExternalInput")
    w2_d = nc.dram_tensor("w2e", (128, 16), dt.bfloat16, kind="ExternalInput")
    ones_d = nc.dram_tensor("onesr", (1, SOB_F), sdt, kind="ExternalInput")
    b2_d = nc.dram_tensor("b2e", (1, 16), dt.float32, kind="ExternalInput")
    out_d = nc.dram_tensor("outp", (SPC, 128, PIX_F), dt.bfloat16, kind="ExternalOutput")

    def eng(name):
        return {"V": nc.vector, "P": nc.gpsimd}[name]

    with tile.TileContext(nc) as tc:
        with (
            tc.tile_pool(name="wpool", bufs=1) as wpool,
            tc.tile_pool(name="xbf", bufs=1) as p_xbf,
            tc.tile_pool(name="pA", bufs=1) as p_A,
            tc.tile_pool(name="pV", bufs=1) as p_V,
            tc.tile_pool(name="pD", bufs=1) as p_D,
            tc.tile_pool(name="xt", bufs=2) as p_xt,
            tc.tile_pool(name="S", bufs=2) as p_S,
            tc.tile_pool(name="hsb", bufs=2) as p_hsb,
            tc.tile_pool(name="small", bufs=2) as p_small,
            tc.tile_pool(name="dx", bufs=2) as p_dx,
            tc.tile_pool(name="pscr", bufs=2) as p_pscr,
            tc.tile_pool(name="vdd", bufs=2, space="DRAM") as p_vdd,
            tc.tile_pool(name="psh", bufs=2, space=bass.MemorySpace.PSUM) as p_psh,
            tc.tile_pool(name="psdx", bufs=1, space=bass.MemorySpace.PSUM) as p_psdx,
        ):
            w1_sb = wpool.tile([KROWS, 128], sdt, tag="w1")
            nc.sync.dma_start(w1_sb[:], w1_d.ap())
            w2_sb = wpool.tile([128, 16], dt.bfloat16, tag="w2")
            nc.sync.dma_start(w2_sb[:], w2_d.ap())
            if b2_nonzero:
                b2_sb = wpool.tile([128, 16], dt.float32, tag="b2")
                nc.sync.dma_start(b2_sb[:], b2_d.ap().broadcast_to([128, 16]))

            # prime both S slots: ones row + edge cols the contiguous
            # shift gathers never write (all persist across slot reuse).
            for _ in range(2):
                St = p_S.tile([KROWS, SOB_F], sdt, tag="S")
                nc.sync.dma_start(St[96:97, :], ones_d.ap())
                nc.vector.memset(St[:, 0:1], 0.0)
                nc.vector.memset(St[:, SOB_F - 1:SOB_F], 0.0)

            def emit_head_loads(s):
                st = {}
                xbf = p_xbf.tile([128, XBF_F], dt.bfloat16, tag="xbf")
                nc.sync.dma_start(xbf[0:64, :], xbf_d.ap()[s, 0:64])
                nc.gpsimd.dma_start(xbf[64:128, :], xbf_d.ap()[s, 64:128])
                xt = p_xt.tile([128, PIX_F], dt.bfloat16, tag="xt")
                nc.gpsimd.dma_start(xt[:], xt_d.ap()[s])
                rt = p_small.tile([128, NT], dt.float32, tag="rt")
                nc.gpsimd.dma_start(rt[:], rt_d.ap()[s])
                xbf3 = xbf.rearrange("p (r q) -> p r q", q=PITCH)
                st.update(xbf3=xbf3, xt=xt, rt=rt,
                          xt3=xt.rearrange("p (t c) -> p t c", c=16))
                return st

            def emit_sobel_a(s, st):
                A = p_A.tile([128, SOB_F], dt.bfloat16, tag="A")
                nc.vector.tensor_tensor(
                    A.rearrange("p (r q) -> p r q", q=PITCH)[:],
                    st["xbf3"][:, 0:32, :], st["xbf3"][:, 2:34, :], op.add)
                st["A"] = A

            def emit_sobel_v(s, st):
                Vt = p_V.tile([128, SOB_F], dt.bfloat16, tag="V")
                nc.vector.scalar_tensor_tensor(
                    Vt.rearrange("p (r q) -> p r q", q=PITCH)[:],
                    st["xbf3"][:, 1:33, :], 2.0,
                    st["A"].rearrange("p (r q) -> p r q", q=PITCH)[:],
                    op.mult, op.add)
                # quarter-stores: strip hb's gather only reads partitions
                # 16hb..16hb+16, so early strips unblock after 1/4 of the
                # (slow, casting) store instead of all of it
                Vd = p_vdd.tile([128, SOB_F], sdt, tag="Vd")
                nc.gpsimd.dma_start(Vd[0:32], Vt[0:32])
                nc.gpsimd.dma_start(Vd[32:64], Vt[32:64])
                st.update(Vd=Vd, Vt=Vt)

            def emit_sobel_d(s, st):
                Dt = p_D.tile([128, SOB_F], dt.bfloat16, tag="D")
                nc.vector.tensor_tensor(
                    Dt.rearrange("p (r q) -> p r q", q=PITCH)[:],
                    st["xbf3"][:, 2:34, :], st["xbf3"][:, 0:32, :],
                    op.subtract)
                Dd = p_vdd.tile([128, SOB_F], sdt, tag="Dd")
                Vd, Vt = st["Vd"], st["Vt"]
                # interleave remaining V and D quarter-stores so strip 0
                # (needs V[0:16]+D[0:16]) unblocks first
                nc.gpsimd.dma_start(Dd[0:32], Dt[0:32])
                nc.gpsimd.dma_start(Vd[64:96], Vt[64:96])
                nc.gpsimd.dma_start(Dd[32:64], Dt[32:64])
                nc.gpsimd.dma_start(Vd[96:128], Vt[96:128])
                nc.gpsimd.dma_start(Dd[64:96], Dt[64:96])
                nc.gpsimd.dma_start(Dd[96:128], Dt[96:128])
                um = p_small.tile([128, NT], dt.bfloat16, tag="um")
                nc.vector.tensor_scalar(um[:], st["rt"][:], FIRE, None,
                                        op.is_lt)
                st.update(Dd=Dd, um=um)

            def emit_head2(s, st):
                alP = p_small.tile([128, NT], dt.bfloat16, tag="alP")
                nc.vector.tensor_copy(alP[:], st["xt3"][:, :, 3])
                preM = p_small.tile([128, NT], dt.bfloat16, tag="preM")
                _pool_and_thresh(nc, p_pscr, alP, preM, op, dt)
                st["preM"] = preM

            def emit_mid(s, st, strips, counters, d_on_sync=False):
                xt, um = st["xt"], st["um"]
                Vd, Dd = st["Vd"], st["Dd"]
                F = SOB_F
                dq = nc.sync if d_on_sync else nc.gpsimd
                for hb in strips:
                    S = p_S.tile([KROWS, SOB_F], sdt, tag="S")
                    pp = slice(16 * hb, 16 * hb + 16)
                    # contiguous DRAM->SBUF shift gathers
                    nc.sync.dma_start(S[0:16, :],
                                      xg_d.ap()[s, pp, PITCH:PITCH + F])
                    nc.sync.dma_start(S[16:32, 1:F], Vd[pp, 0:F - 1])
                    nc.sync.dma_start(S[32:48, 0:F - 1], Vd[pp, 1:F])
                    dq.dma_start(S[48:64, 1:F], Dd[pp, 0:F - 1])
                    nc.sync.dma_start(S[64:80, :], Dd[pp, :])
                    dq.dma_start(S[80:96, 0:F - 1], Dd[pp, 1:F])

                    hsb = p_hsb.tile([128, SOB_F], dt.bfloat16, tag="hsb")
                    psdx = p_psdx.tile([128, 1024], dt.float32, tag="psdx")

                    def emit_mm2(limit):
                        # mm2 for pixel tiles whose hsb window is fully
                        # evacuated (off+128 <= limit)
                        while True:
                            t = counters["t_next"]
                            if t >= 64:
                                break
                            off = (t // 2) * PITCH + 1 + (t % 2) * 128
                            if off + 128 > limit:
                                break
                            nc.tensor.matmul(
                                psdx[:, 16 * t:16 * t + 16],
                                hsb[:, off:off + 128],
                                w2_sb[:])
                            counters["t_next"] += 1

                    # software-pipelined by two chunks: mm2 of chunk i-2
                    # is emitted after mm1 of chunk i, so the PE never
                    # waits in-order on a just-triggered evac.
                    ends = [0, 0]
                    for (c0, cw) in CHUNKS:
                        psh = p_psh.tile([128, 1536], dt.float32, tag="psh")
                        nmm = (cw + 511) // 512
                        for j in range(nmm):
                            w = min(512, cw - 512 * j)
                            nc.tensor.matmul(
                                psh[:, 512 * j:512 * j + w],
                                w1_sb[:],
                                S[:, c0 + 512 * j:c0 + 512 * j + w])
                        emit_mm2(ends[-2])
                        ends.append(c0 + cw)
                        ci = counters["chunk"]
                        counters["chunk"] += 1
                        if ci % EVAC_V_EVERY == EVAC_V_EVERY - 1:
                            if S_FP8:
                                nc.vector.tensor_scalar(
                                    hsb[:, c0:c0 + cw], psh[:, :cw],
                                    0.125, 0.0, op.mult, op.max)
                            else:
                                nc.vector.tensor_scalar(
                                    hsb[:, c0:c0 + cw], psh[:, :cw],
                                    0.0, None, op.max)
                        else:
                            nc.scalar.activation(
                                hsb[:, c0:c0 + cw], psh[:, :cw], AF.Relu,
                                scale=0.125 if S_FP8 else 1.0)
                    emit_mm2(ends[-2])
                    emit_mm2(SOB_F)
                    counters["t_next"] = 0
                    _evac_strip(nc, psdx, hb, um, xt, op, dt, p_dx,
                                b2_sb if b2_nonzero else None)

            def emit_tail(s, st):
                xt, xt3 = st["xt"], st["xt3"]
                alN = p_small.tile([128, NT], dt.bfloat16, tag="alN")
                nc.vector.tensor_copy(alN[:], xt3[:, :, 3])
                postM = p_small.tile([128, NT], dt.bfloat16, tag="postM")
                _pool_and_thresh(nc, p_pscr, alN, postM, op, dt)
                life = p_small.tile([128, NT], dt.bfloat16, tag="life")
                nc.vector.tensor_tensor(life[:], st["preM"][:], postM[:],
                                        op.mult)
                nh = len(MULT_PAT)
                tq = NT // nh
                for q in range(nh):
                    ts = slice(tq * q, tq * (q + 1))
                    cs = slice(16 * tq * q, 16 * tq * (q + 1))
                    eng(MULT_PAT[q]).tensor_tensor(
                        xt3[:, ts, :], xt3[:, ts, :],
                        life[:, ts].broadcast_to([128, tq, 16]), op.mult)
                    nc.gpsimd.dma_start(out_d.ap()[s][:, cs], xt[:, cs])

            counters = {"chunk": 0, "t_next": 0}
            st0 = emit_head_loads(0)
            emit_sobel_a(0, st0)
            emit_sobel_v(0, st0)
            emit_sobel_d(0, st0)
            emit_head2(0, st0)
            emit_mid(0, st0, range(0, 3), counters)
            st1 = emit_head_loads(1)
            emit_sobel_a(1, st1)
            emit_mid(0, st0, range(3, 4), counters)
            emit_sobel_v(1, st1)
            emit_mid(0, st0, range(4, 5), counters)
            emit_sobel_d(1, st1)
            emit_mid(0, st0, range(5, 8), counters)
            emit_head2(1, st1)
            emit_tail(0, st0)
            emit_mid(1, st1, range(0, 8), counters)
            emit_tail(1, st1)

    nc.compile()
    return nc


def _evac_strip(nc, psdx, hb, um, xt, op, dt, p_dx, b2_sb):
    """Strip hb (8192 px, 64 tiles): dx*um and x += in pixel-major."""
    ps3 = psdx.rearrange("p (t c) -> p t c", c=16)
    umk = um[:, 64 * hb:64 * hb + 64]
    sl = slice(1024 * hb, 1024 * (hb + 1))
    if b2_sb is not None:
        nc.vector.tensor_tensor(
            ps3[:], ps3[:],
            b2_sb[:].rearrange("p c -> p 1 c").broadcast_to([128, 64, 16]),
            op.add)
    DXM = p_dx.tile([128, 1024], dt.bfloat16, tag="DXM")
    nc.vector.tensor_tensor(
        DXM.rearrange("p (t c) -> p t c", c=16), ps3[:],
        umk.broadcast_to([128, 64, 16]), op.mult)
    nc.vector.tensor_tensor(xt[:, sl], xt[:, sl], DXM[:], op.add)


def _pool_and_thresh(nc, pool, alpha, outM, op, dt):
    """3x3 circular max-pool on pixel-major alpha [128, NT] then > ALPHA_TH.

    Neighbor tensors staged entirely by DMA, then one straight-line
    Vector block.
    """
    f16 = dt.bfloat16
    aL = pool.tile([128, NT], f16, tag="aL")
    aR = pool.tile([128, NT], f16, tag="aR")
    nc.sync.dma_start(aL[1:128, :], alpha[0:127, :])
    nc.gpsimd.dma_start(aR[0:127, :], alpha[1:128, :])
    nc.sync.dma_start(aL[0:1, 0:NT:2], alpha[127:128, 1:NT:2])
    nc.sync.dma_start(aL[0:1, 1:NT:2], alpha[127:128, 0:NT - 1:2])
    nc.gpsimd.dma_start(aR[127:128, 0:NT:2], alpha[0:1, 1:NT:2])
    nc.gpsimd.dma_start(aR[127:128, 1:NT:2], alpha[0:1, 0:NT - 1:2])
    PW = pool.tile([128, NT], f16, tag="PW")
    nc.vector.tensor_tensor(PW[:], alpha[:, :], aL[:], op.max)
    nc.vector.tensor_tensor(PW[:], PW[:], aR[:], op.max)
    z2 = pool.tile([128, NT], f16, tag="z2")
    nc.vector.tensor_tensor(z2[:, 0:NT - 2], PW[:, 0:NT - 2], PW[:, 2:NT], op.max)
    nc.vector.tensor_tensor(outM[:, 2:NT - 2], z2[:, 0:NT - 4], PW[:, 4:NT], op.max)
    nc.vector.tensor_tensor(outM[:, 0:2], z2[:, 0:2], PW[:, NT - 2:NT], op.max)
    nc.vector.tensor_tensor(outM[:, NT - 2:NT], z2[:, NT - 4:NT - 2],
                            PW[:, 0:2], op.max)
    nc.vector.tensor_scalar(outM[:], outM[:], ALPHA_TH, None, op.is_gt)


def _get_built(b2_nonzero):
    global _BUILT
    if _BUILT is None or _BUILT[0] != b2_nonzero:
        _BUILT = (b2_nonzero, _build(b2_nonzero))
    return _BUILT[1]


# ------------------------------------------------------------------ kernel
def kernel(x, rand_vals, w1, b1, w2, b2):
    from concourse.bass_utils import run_bass_kernel_spmd

    x = np.asarray(x, np.float32)
    rand_vals = np.asarray(rand_vals, np.float32)
    w1e, w2e, b2e = _prep_weights(w1, b1, w2, b2)
    b2_nonzero = bool(np.any(b2e != 0.0))

    xbf = _prep_xbf(x, _bf16())
    xt = _prep_xt(x)
    rt = _prep_randt(rand_vals)
    sdt_np = _fp8() if S_FP8 else _bf16()
    ones = np.ones((1, SOB_F), dtype=sdt_np)

    nc = _get_built(b2_nonzero)

    in_maps = []
    for i in range(NCORES):
        sl = slice(SPC * i, SPC * (i + 1))
        m = {
            "xbf": np.ascontiguousarray(xbf[sl]),
            "xt": np.ascontiguousarray(xt[sl]),
            "rt": np.ascontiguousarray(rt[sl]),
            "w1e": w1e, "w2e": w2e, "onesr": ones,
            "b2e": b2e.reshape(1, 16),
        }
        if S_FP8:
            m["xf8"] = np.ascontiguousarray(
                _prep_xbf(x, _fp8())[sl])
        in_maps.append(m)

    res = run_bass_kernel_spmd(nc, in_maps, core_ids=list(range(NCORES)))
    outs = [res.results[i]["outp"] for i in range(NCORES)]
    out_pm = np.concatenate(outs, axis=0)
    return _unprep_out(out_pm)
